# revision 15
# baseline (speedup 1.0000x reference)
"""ResNet bottleneck block (1x1 -> 3x3 -> 1x1 convs, folded BN, residual ReLU)
on 8 Trainium2 NeuronCores, data-parallel over the batch dim.

fp8 strategy (default, MM_MODE=fp8):
  - All three convs run as fp8e4m3 DoubleRow matmuls: each instruction
    contracts K=256 (two 128-chunks packed in free dim 1 of both operands)
    at 2x the bf16 rate -> ~68 matmuls/image instead of 136.
  - BN is folded into the weights host-side; per-channel static scales
    (derived from weight norms, no activation peeking) keep activations in
    fp8 range.  Scale/shift ride the ScalarE PSUM->SBUF evacuation
    (relu(ps*scale+bias)) for free; activations are stored fp8.
  - x comes in twice: fp8 copy for conv1, bf16 copy for the residual add.
    Output is stored bf16 and upcast on host (halves in/out DMA traffic).
  - conv3 epilogue: VectorE stt computes ps*(1/s_w3)+x, ScalarE applies
    relu(.+sh3) casting to bf16.
  - Software pipeline over images: DMA(t) / conv1(t-1) / conv2(t-2) /
    conv3+store(t-3).

MM_MODE=fp8b: conv1 runs in bf16 off the residual copy of x (no fp8 x
copy at all) for extra accuracy margin; conv2/conv3 stay fp8 DoubleRow.
MM_MODE=f32r keeps the old full-precision path.
"""

import math
import os

import numpy as np
import ml_dtypes

import concourse.bass as bass
import concourse.mybir as mybir
import concourse.tile as tile
from concourse.bass_utils import run_bass_kernel_spmd

# Problem constants (hardcoded per the grading contract).
B, CIN, H, W = 64, 1024, 28, 28
WIDTH, COUT = 256, 1024
NCORES = 8
BPC = B // NCORES          # images per core
S = H * W                  # 784
PW = W + 2                 # 30 (padded row width)
PS = PW * PW               # 900
NROW = H // 2              # 14 rows per spatial chunk
NS = NROW * W              # 392 columns per matmul
P = 128
KC_IN = CIN // P           # 8
KC4 = CIN // 256           # 4 DoubleRow K-groups for conv1
KT = 2                     # k-tiles per DoubleRow matmul
MC_W = WIDTH // P          # 2
MC_OUT = COUT // P         # 8
EPS = 1e-5

F32 = mybir.dt.float32
F8 = mybir.dt.float8e4
BF16 = mybir.dt.bfloat16
NP_F8 = ml_dtypes.float8_e4m3      # TRN fp8e4: max normal 240
NP_BF16 = ml_dtypes.bfloat16
Relu = mybir.ActivationFunctionType.Relu
ADD = mybir.AluOpType.add
MULT = mybir.AluOpType.mult
DR = mybir.MatmulPerfMode.DoubleRow

F8_MAX = 224.0   # target absmax when scaling into fp8 (240 is the cap)

MM_MODE = os.environ.get("KERNEL_MM_MODE", "fp8")

_NC_CACHE = {}
LAST_RESULT = None  # test.py reads exec_time_ns off this


def _split_multi_waits(nc, maxw=1):
    """walrus codegen rejects instructions carrying more than a couple of
    sem waits ("Too many sync wait commands"); hoist excess waits onto
    same-engine NOPs emitted just before the instruction."""
    for f in nc.m.functions:
        for blk in f.blocks:
            out = []
            changed = False
            for inst in blk.instructions:
                si = inst.sync_info
                if si is not None and len(si.on_wait) > maxw:
                    waits = list(si.on_wait)
                    head, keep = waits[:-maxw], waits[-maxw:]
                    for i in range(0, len(head), maxw):
                        nop = mybir.InstNoOp(
                            name=f"{inst.name}_waitsplit_{i}", ins=[], outs=[]
                        )
                        nop.engine = inst.engine
                        nop.sync_info = mybir.SyncInfo(
                            on_wait=head[i:i + maxw], on_update=[]
                        )
                        out.append(nop)
                    inst.sync_info = mybir.SyncInfo(
                        on_wait=keep, on_update=list(si.on_update)
                    )
                    changed = True
                out.append(inst)
            if changed:
                blk.instructions = out


def _build_nc_fp8(conv1_bf16):
    nc = bass.Bass()
    if not conv1_bf16:
        x8_d = nc.dram_tensor("x8", [BPC, P, KC4, KT, S], F8, kind="ExternalInput")
        w1_d = nc.dram_tensor("w1", [P, KC4, KT, MC_W, P], F8, kind="ExternalInput")
    else:
        w1_d = nc.dram_tensor("w1", [P, KC_IN, MC_W, P], BF16, kind="ExternalInput")
    xr_d = nc.dram_tensor("xr", [BPC, P, MC_OUT, S], BF16, kind="ExternalInput")
    w2_d = nc.dram_tensor("w2", [P, KT, 9, MC_W, P], F8, kind="ExternalInput")
    w3_d = nc.dram_tensor("w3", [P, KT, MC_OUT, P], F8, kind="ExternalInput")
    sc1_d = nc.dram_tensor("sc1", [P, MC_W], F32, kind="ExternalInput")
    b1_d = nc.dram_tensor("b1", [P, MC_W], F32, kind="ExternalInput")
    sc2_d = nc.dram_tensor("sc2", [P, MC_W], F32, kind="ExternalInput")
    b2_d = nc.dram_tensor("b2", [P, MC_W], F32, kind="ExternalInput")
    sc3_d = nc.dram_tensor("sc3", [P, MC_OUT], F32, kind="ExternalInput")
    o_d = nc.dram_tensor("o", [BPC, MC_OUT, P, S], BF16, kind="ExternalOutput")
    dbg = os.environ.get("KERNEL_DEBUG_TAPS") == "1"
    if dbg:
        da1_d = nc.dram_tensor("da1", [P, KT, PS + 2], F8, kind="ExternalOutput")
        da2_d = nc.dram_tensor("da2", [P, KT, S], F8, kind="ExternalOutput")

    with tile.TileContext(nc) as tc:
        with (
            tc.tile_pool(name="consts", bufs=1) as cpool,
            tc.tile_pool(name="x8p", bufs=3) as x8pool,
            tc.tile_pool(name="xrp", bufs=5) as xrpool,
            tc.tile_pool(name="a2p", bufs=2) as a2pool,
            tc.tile_pool(name="otp", bufs=4) as opool,
            tc.tile_pool(name="ttp", bufs=4) as tpool,
            tc.tile_pool(name="psp", bufs=2, space="PSUM") as pspool,
        ):
            if not conv1_bf16:
                w1_sb = cpool.tile([P, KC4, KT, MC_W, P], F8, tag="w1")
            else:
                w1_sb = cpool.tile([P, KC_IN, MC_W, P], BF16, tag="w1")
            w2_sb = cpool.tile([P, KT, 9, MC_W, P], F8, tag="w2")
            w3_sb = cpool.tile([P, KT, MC_OUT, P], F8, tag="w3")
            sc1_sb = cpool.tile([P, MC_W], F32, tag="sc1")
            b1_sb = cpool.tile([P, MC_W], F32, tag="b1")
            sc2_sb = cpool.tile([P, MC_W], F32, tag="sc2")
            b2_sb = cpool.tile([P, MC_W], F32, tag="b2")
            sc3_sb = cpool.tile([P, MC_OUT], F32, tag="sc3")

            # a1 lives in two fixed buffers; the zero pad border is painted
            # once here and only the interior is rewritten per image.  The
            # flat free dim is PS+2 because the (dy=2,dx=2) tap's contiguous
            # 420-run reads 2 elements past the 30x30 image.
            a1_bufs = [
                cpool.tile([P, KT, PS + 2], F8, tag=f"a1_{i}", name=f"a1_{i}")
                for i in range(2)
            ]
            for a1b in a1_bufs:
                a1v = a1b[:, :, :PS].rearrange("p k (r c) -> p k r c", c=PW)
                nc.vector.memset(a1v[:, :, 0, :], 0.0)
                nc.vector.memset(a1v[:, :, PW - 1, :], 0.0)
                nc.vector.memset(a1v[:, :, 1:PW - 1, 0], 0.0)
                nc.vector.memset(a1v[:, :, 1:PW - 1, PW - 1], 0.0)
                nc.vector.memset(a1b[:, :, PS:], 0.0)

            # Pre-warm the PE during the DMA lead-in: HAM starts the PE
            # throttled at 1.2 GHz and needs ~3.4us of sustained activity to
            # un-gate; ~5us of dummy matmuls (no DMA dependency) get that out
            # of the way before the first real matmul's operands land.
            warm_sb = cpool.tile([P, P], BF16, tag="warm")
            nc.vector.memset(warm_sb[:], 0.0)
            for _ in range(64):
                wps = pspool.tile([P, 64], F32, tag="ps1", name="wps")
                nc.tensor.matmul(wps[:], warm_sb[:], warm_sb[:, :64],
                                 start=True, stop=True)

            x8s = {}     # t -> fp8 [P, KC4, KT, S] tile (fp8 conv1 input)
            xrs = {}     # t -> bf16 [P, MC_OUT, S] tile (residual / bf16 conv1)
            a2s = {}     # t -> fp8 [P, KT, S] tile

            def load(t, xr_only=False):
                if not conv1_bf16 and not xr_only:
                    xf = x8pool.tile([P, KC4, KT, S], F8, tag="x8")
                    for kc4 in range(KC4):
                        nc.sync.dma_start(xf[:, kc4], x8_d[t, :, kc4])
                    x8s[t] = xf
                xr = xrpool.tile([P, MC_OUT, S], BF16, tag="xr")
                for h in range(4):
                    nc.sync.dma_start(
                        xr[:, 2 * h:2 * h + 2], xr_d[t, :, 2 * h:2 * h + 2]
                    )
                xrs[t] = xr

            def conv1_mc(t, mc):
                a1b = a1_bufs[t % 2][:, :, :PS].rearrange(
                    "p k (r c) -> p k r c", c=PW)
                pss = [pspool.tile([P, NS], F32, tag="ps1", name="ps1")
                       for _ in range(2)]
                if conv1_bf16:
                    xr = xrs[t]
                    for kc in range(KC_IN):
                        for sc in range(2):
                            ns = slice(sc * NS, (sc + 1) * NS)
                            mm = nc.tensor.matmul(
                                pss[sc][:], w1_sb[:, kc, mc], xr[:, kc, ns],
                                start=(kc == 0), stop=(kc == KC_IN - 1),
                            )
                            if sc == 1:
                                mm.ldweights = False
                else:
                    xf = x8s[t]
                    for kc4 in range(KC4):
                        for sc in range(2):
                            ns = slice(sc * NS, (sc + 1) * NS)
                            mm = nc.tensor.matmul(
                                pss[sc][:], w1_sb[:, kc4, :, mc],
                                xf[:, kc4, :, ns],
                                start=(kc4 == 0), stop=(kc4 == KC4 - 1),
                                perf_mode=DR,
                            )
                            if sc == 1:
                                mm.ldweights = False
                for sc in range(2):
                    r0 = sc * NROW
                    psr = pss[sc].rearrange("p (r c) -> p r c", c=W)
                    nc.scalar.activation(
                        a1b[:, mc, 1 + r0:1 + r0 + NROW, 1:1 + W],
                        psr,
                        Relu,
                        bias=b1_sb[:, mc:mc + 1],
                        scale=sc1_sb[:, mc:mc + 1],
                    )
                if not conv1_bf16 and mc == MC_W - 1:
                    del x8s[t]

            def conv2_mc(t, mc):
                # Each tap reads one contiguous 420-column run (14 rows x
                # 30-wide padded stride) so the moving AP stays 3-dim; all 9
                # taps map output (jr, x) to the same psum column jr*30+x,
                # and columns with col%30 in {28, 29} are junk the strided
                # evacuation skips.  The second spatial half reuses the tap
                # weights already in the PE array (ldweights=False).
                a1f = a1_bufs[t % 2]
                if mc == 0:
                    a2s[t] = a2pool.tile([P, KT, S], F8, tag="a2", name="a2")
                a2 = a2s[t]
                pss = [pspool.tile([P, NROW * PW], F32, tag="ps2", name="ps2")
                       for _ in range(2)]
                for d in range(9):
                    dy, dx = d // 3, d % 3
                    for sc in range(2):
                        off = (sc * NROW + dy) * PW + dx
                        mm = nc.tensor.matmul(
                            pss[sc][:],
                            w2_sb[:, :, d, mc],
                            a1f[:, :, off:off + NROW * PW],
                            start=(d == 0),
                            stop=(d == 8),
                            perf_mode=DR,
                        )
                        if sc == 1:
                            mm.ldweights = False
                for sc in range(2):
                    psr = pss[sc].rearrange("p (r c) -> p r c", c=PW)
                    a2r = a2[:, mc, sc * NS:(sc + 1) * NS].rearrange(
                        "p (r c) -> p r c", c=W)
                    nc.scalar.activation(
                        a2r,
                        psr[:, :, :W],
                        Relu,
                        bias=b2_sb[:, mc:mc + 1],
                        scale=sc2_sb[:, mc:mc + 1],
                    )

            def conv3_mc(t, mc):
                a2 = a2s[t]
                xr = xrs[t]
                osb = opool.tile([P, S], BF16, tag="osb", name="osb")
                tt = tpool.tile([P, S], F32, tag="tt", name="tt")
                pss = [pspool.tile([P, NS], F32, tag="ps3", bufs=4, name="ps3")
                       for _ in range(2)]
                for sc in range(2):
                    ns = slice(sc * NS, (sc + 1) * NS)
                    mm = nc.tensor.matmul(
                        pss[sc][:], w3_sb[:, :, mc], a2[:, :, ns],
                        start=True, stop=True, perf_mode=DR,
                    )
                    if sc == 1:
                        mm.ldweights = False
                for sc in range(2):
                    ns = slice(sc * NS, (sc + 1) * NS)
                    nc.vector.scalar_tensor_tensor(
                        tt[:, ns], pss[sc][:], sc3_sb[:, mc:mc + 1],
                        xr[:, mc, ns], MULT, ADD,
                    )
                nc.gpsimd.tensor_scalar_max(osb[:], tt[:], 0.0)
                nc.sync.dma_start(o_d[t, mc], osb[:])
                if mc == MC_OUT - 1:
                    del a2s[t], xrs[t]

            # Startup DMA order: scales first (tiny; ScalarE's first evac
            # needs them), then w1/x(0) chunks interleaved so the PE can
            # start as soon as the first K-group lands, w2 right behind so
            # conv2(0) isn't starved.
            if not conv1_bf16:
                xf0 = x8pool.tile([P, KC4, KT, S], F8, tag="x8", name="x8f")
                nc.sync.dma_start(w1_sb[:, 0], w1_d[:, 0])
                nc.sync.dma_start(xf0[:, 0], x8_d[0, :, 0])
            for sb, d in [(sc1_sb, sc1_d), (b1_sb, b1_d), (sc2_sb, sc2_d),
                          (b2_sb, b2_d), (sc3_sb, sc3_d)]:
                nc.sync.dma_start(sb[:], d[:])
            if not conv1_bf16:
                for kc4 in range(KC4):
                    if kc4 > 0:
                        nc.sync.dma_start(w1_sb[:, kc4], w1_d[:, kc4])
                        nc.sync.dma_start(xf0[:, kc4], x8_d[0, :, kc4])
                    nc.sync.dma_start(w2_sb[:, :, 2 * kc4], w2_d[:, :, 2 * kc4])
                    if kc4 < 3:
                        nc.sync.dma_start(
                            w2_sb[:, :, 2 * kc4 + 1], w2_d[:, :, 2 * kc4 + 1]
                        )
                x8s[0] = xf0
                nc.sync.dma_start(w2_sb[:, :, 7:9], w2_d[:, :, 7:9])
                xr0 = xrpool.tile([P, MC_OUT, S], BF16, tag="xr", name="xr0")
                for h in range(4):
                    nc.sync.dma_start(xr0[:, 2 * h:2 * h + 2],
                                      xr_d[0, :, 2 * h:2 * h + 2])
                xrs[0] = xr0
            else:
                xr0 = xrpool.tile([P, MC_OUT, S], BF16, tag="xr", name="xr0")
                for kc in range(KC_IN):
                    nc.sync.dma_start(w1_sb[:, kc], w1_d[:, kc])
                    nc.sync.dma_start(xr0[:, kc], xr_d[0, :, kc])
                xrs[0] = xr0
                nc.sync.dma_start(w2_sb[:], w2_d[:])
            nc.sync.dma_start(w3_sb[:], w3_d[:])
            load(1)

            for t in range(BPC + 3):
                if 1 < t < BPC:
                    load(t)
                # Interleave conv3 chunks between conv1/conv2 groups so the
                # in-order PE stream never stalls behind the DVE/ScalarE
                # epilogue chain of conv3.
                units = []
                if 0 <= t - 1 < BPC:
                    units += [("c1", (t - 1, mc)) for mc in range(MC_W)]
                if 0 <= t - 2 < BPC:
                    units += [("c2", (t - 2, mc)) for mc in range(MC_W)]
                c3u = ([("c3", (t - 3, mc)) for mc in range(MC_OUT)]
                       if 0 <= t - 3 < BPC else [])
                out_units = []
                nu = max(len(units), 1)
                k = 0
                for i, u in enumerate(units):
                    out_units.append(u)
                    want = (i + 1) * len(c3u) // nu
                    while k < want:
                        out_units.append(c3u[k])
                        k += 1
                out_units += c3u[k:]
                for kind, args in out_units:
                    if kind == "c1":
                        conv1_mc(*args)
                    elif kind == "c2":
                        conv2_mc(*args)
                    else:
                        conv3_mc(*args)
                if dbg and t == 1:
                    nc.sync.dma_start(da1_d[:], a1_bufs[0][:])
                if dbg and t == 2:
                    nc.sync.dma_start(da2_d[:], a2s[0][:])

    return nc


def _fold(wv, g, bb, m, v):
    inv = (g / np.sqrt(np.asarray(v, np.float32) + EPS)).astype(np.float32)
    shift = (np.asarray(bb, np.float32) - np.asarray(m, np.float32) * inv)
    return np.asarray(wv, np.float32) * inv[:, None, None, None], shift


def _q8(a):
    return np.clip(np.asarray(a, np.float32), -240.0, 240.0).astype(NP_F8)


def _prep_inputs_fp8(conv1_bf16, x, w1, w2, w3, g1, b1, m1, v1,
                     g2, b2, m2, v2, g3, b3, m3, v3):
    w1f, sh1 = _fold(w1, g1, b1, m1, v1)
    w1f = w1f[:, :, 0, 0]                      # [256, 1024]
    w2f, sh2 = _fold(w2, g2, b2, m2, v2)       # [256, 256, 3, 3]
    w3f, sh3 = _fold(w3, g3, b3, m3, v3)
    w3f = w3f[:, :, 0, 0]                      # [1024, 256]

    # Static fp8 ranges from weight norms (x ~ N(0,1) per channel):
    #   a1_c ~ relu(N(sh1_c, ||w1f_c||^2))
    n1 = np.linalg.norm(w1f, axis=1)
    sa1 = F8_MAX / (np.abs(sh1) + 6.0 * n1)
    # E[a1_c^2] <= sh1_c^2 + ||w1f_c||^2 (pre-relu second moment bound)
    a1_m2 = sh1 ** 2 + n1 ** 2
    w2s = w2f / sa1[None, :, None, None]
    var2 = ((w2f ** 2) * a1_m2[None, :, None, None]).sum(axis=(1, 2, 3))
    sa2 = F8_MAX / (np.abs(sh2) + 6.0 * np.sqrt(var2))
    w3s = w3f / sa2[None, :]

    s_w2 = F8_MAX / np.abs(w2s).max()
    s_w3 = F8_MAX / np.abs(w3s).max()

    # lhsT layouts
    if not conv1_bf16:
        s_w1 = F8_MAX / np.abs(w1f).max()
        w1h = _q8(np.ascontiguousarray(
            (w1f * s_w1).T.reshape(KC4, KT, P, MC_W, P).transpose(2, 0, 1, 3, 4)
        ))
        sc1h = (sa1 / s_w1).reshape(MC_W, P).T
    else:
        w1h = np.ascontiguousarray(
            w1f.T.reshape(KC_IN, P, MC_W, P).transpose(1, 0, 2, 3)
        ).astype(NP_BF16)
        sc1h = sa1.reshape(MC_W, P).T
    w2h = _q8(np.ascontiguousarray(
        (w2s * s_w2).transpose(1, 2, 3, 0)        # [in, ky, kx, out]
        .reshape(KT, P, 9, MC_W, P)
        .transpose(1, 0, 2, 3, 4)
    ))
    w3h = _q8(np.ascontiguousarray(
        (w3s * s_w3).T.reshape(KT, P, MC_OUT, P).transpose(1, 0, 2, 3)
    ))
    b1h = (sh1 * sa1).reshape(MC_W, P).T
    sc2h = (sa2 / s_w2).reshape(MC_W, P).T
    b2h = (sh2 * sa2).reshape(MC_W, P).T
    sc3h = np.full((P, MC_OUT), 1.0 / s_w3, np.float32)

    xnp = np.asarray(x, np.float32).reshape(B, COUT, S)
    xr = np.ascontiguousarray(
        (xnp + sh3[None, :, None])
        .reshape(B, MC_OUT, P, S).transpose(0, 2, 1, 3)
    ).astype(NP_BF16)
    if not conv1_bf16:
        x8 = np.ascontiguousarray(
            xnp.reshape(B, KC4, KT, P, S).transpose(0, 3, 1, 2, 4)
        ).astype(NP_F8)

    common = {
        "w1": w1h, "w2": w2h, "w3": w3h,
        "sc1": np.ascontiguousarray(sc1h), "b1": np.ascontiguousarray(b1h),
        "sc2": np.ascontiguousarray(sc2h), "b2": np.ascontiguousarray(b2h),
        "sc3": sc3h,
    }
    in_maps = []
    for c in range(NCORES):
        m = dict(common)
        m["xr"] = np.ascontiguousarray(xr[c * BPC:(c + 1) * BPC])
        if not conv1_bf16:
            m["x8"] = np.ascontiguousarray(x8[c * BPC:(c + 1) * BPC])
        in_maps.append(m)
    return in_maps


def _ensure_ntff_hook():
    """If tracing is requested but this image's antenv lacks axon_hooks,
    register an in-process shim (or disable tracing) so run_bass_kernel_spmd
    doesn't crash on the import."""
    if os.environ.get("BASS_TRACE") != "1":
        return
    try:
        import antenv.axon_hooks  # noqa: F401
        return
    except ImportError:
        pass
    try:
        import sys
        import types
        import antenv
        from trn_agent_boot.trn_boot import _ntff_profile_via_ctypes

        hook = _ntff_profile_via_ctypes("/opt/axon/libaxon_pjrt.so")
        mod = types.ModuleType("antenv.axon_hooks")
        state = {"hook": hook}
        mod.set_axon_ntff_profile_hook = lambda h: state.__setitem__("hook", h)
        mod.get_axon_ntff_profile_hook = lambda: state["hook"]
        antenv.axon_hooks = mod
        sys.modules["antenv.axon_hooks"] = mod
    except Exception:
        os.environ["BASS_NEVER_TRACE"] = "1"


def kernel(**inputs):
    global LAST_RESULT
    _ensure_ntff_hook()
    if MM_MODE not in _NC_CACHE:
        nc = _build_nc_fp8(conv1_bf16=(MM_MODE == "fp8b"))
        _split_multi_waits(nc)  # HW-only legalization; CoreSim can't run it
        _NC_CACHE[MM_MODE] = nc
    nc = _NC_CACHE[MM_MODE]
    in_maps = _prep_inputs_fp8(MM_MODE == "fp8b", **inputs)
    res = run_bass_kernel_spmd(nc, in_maps, list(range(NCORES)))
    LAST_RESULT = res
    out = np.concatenate(
        [np.asarray(r["o"], np.float32) for r in res.results], axis=0
    )
    return np.ascontiguousarray(out.reshape(B, COUT, H, W))
